# revision 32
# baseline (speedup 1.0000x reference)
"""Fused transformer encoder layer (pre-LN, MHA + SiLU FFN) for Trainium2,
data-parallel over (batch, query-half) across 8 NeuronCores.

Contract: kernel(**inputs) takes the FULL unsharded inputs (numpy arrays, as
produced by the problem's setup_inputs) and returns the FULL [B, S, D] fp32
output. Sharding: core c handles batch b = c // 2 and query half h = c % 2
(1024 queries); each core computes K/V over the full 2048-token sequence of
its batch (keys/values are order-invariant under softmax, so each core gets
its x rotated to put its own queries first — one uniform SPMD program).

Key structure (all matmul operands bf16, fp32 PSUM accumulation, fp32
residual path):
  - LN1/LN2: DVE bn_stats + Newton rsqrt; affine as DVE tensor_scalar with
    per-partition scalar APs (frees ScalarE for softmax exp).
  - QKV^T: K-bias dropped exactly (constant-per-query shift cancels in
    softmax); V-bias folded into b_proj on host; Q-bias applied on the
    ScalarE Identity evacuation.
  - Attention processes query chunks of 256 so the score tile is 2 PSUM
    banks and can be double-buffered (sc0/sc1); softmax exp alternates
    between ScalarE (true Exp) and VectorE (Schraudolph bit-trick exp:
    bf16 bits = int16(x*128/ln2 + (127*128-5.76)); ~1.8% mean rel err that
    largely cancels inside softmax). Both engines stream concurrently with
    the PE score/AV/den matmuls -> no engine serialization point.
  - Softmax normalization is deferred out of the loop: denominators
    accumulate per-block in PSUM, are transpose-compacted to [128, 64],
    one cheap VectorE reciprocal, transposed back, and applied via a K=4
    broadcast matmul + one tensor_tensor multiply per block.
"""

import os
import sys

for _p in ("/opt/trn_rl_repo", "/root/.axon_site/_ro/trn_rl_repo"):
    if os.path.isdir(_p) and _p not in sys.path:
        sys.path.insert(0, _p)

import numpy as np
import ml_dtypes

import concourse.bass as bass
import concourse.tile as tile
from concourse import mybir
from concourse.bass_utils import run_bass_kernel_spmd
from concourse.vector_clock import ScopedClock

BF16 = ml_dtypes.bfloat16
F32 = mybir.dt.float32
BF = mybir.dt.bfloat16
I16 = mybir.dt.int16

B, S, D = 4, 2048, 256
H, DH = 8, 32
DFF = 2 * D
SQ = S // 2          # queries per core
NQT = SQ // 128      # query token tiles per core (8)
NKT = S // 128       # key token tiles (16)
EPS = 1e-5
AF = mybir.ActivationFunctionType
ALU = mybir.AluOpType

# Schraudolph fast-exp constants (bf16 target, verified on HW:
# mean rel 1.8%, max 3.3% for |x| <~ 5)
EXP_A = 128.0 / np.log(2.0)
EXP_B = 127.0 * 128.0 - 0.0450 * 128.0


def _split_excess_waits(nc, max_waits=1):
    """This walrus build lowers at most one sem wait per TPB instruction
    ("Too many sync wait commands" otherwise, matching bass's own
    inst_waits_full model). Move excess waits onto same-engine NoOps
    inserted directly before the instruction — the engine queue executes
    them in order, so the barrier semantics are unchanged."""
    cnt = 0
    for f in nc.m.functions:
        for bb in f.blocks:
            new = []
            changed = False
            for ins in bb.instructions:
                si = getattr(ins, "sync_info", None)
                waits = list(si.on_wait) if si is not None else []
                if len(waits) > max_waits:
                    for w in waits[:-max_waits]:
                        nop = mybir.InstNoOp(name=f"wsplit-{cnt}", ins=[], outs=[])
                        cnt += 1
                        nop.engine = ins.engine
                        nop.sync_info = mybir.SyncInfo(on_wait=[w], on_update=[])
                        new.append(nop)
                    ins.sync_info = mybir.SyncInfo(
                        on_wait=waits[-max_waits:], on_update=list(si.on_update))
                    changed = True
                new.append(ins)
            if changed:
                bb.instructions = new


def _patch_tile_drain():
    """walrus in this container rejects >~2 sem waits on the SP tail drain
    ("Too many sync wait commands"); emit one drain per clock proc instead."""

    def _drain_and_barrier(self, tick_clock, wait_clock):
        vclock = tick_clock.global_clock
        for proc in range(len(vclock)):
            t = vclock[proc]
            if t > 0:
                d = self.nc.sync.drain()
                part = ScopedClock()
                part.require_at_least(None, proc, t)
                wait_clock.add_sem_waits(d.ins, part)
        self.nc.all_engine_barrier()
        assert self.sems is not None
        popped = self.nc._tile_sem_poison_stack.pop()
        assert popped is self._sem_poison
        self.nc.clear_and_free_semaphores(list(self.sems.allocated().values()))
        self.nc.all_engine_barrier()

    tile.TileContext._drain_and_barrier = _drain_and_barrier


def _newton_rsqrt(nc, pool, veps, nt, iters=3):
    """y ~= 1/sqrt(veps) on DVE only (ACT Rsqrt is banned for accuracy).
    Seed y0 = 1/veps converges for veps in (0, 3). veps here is ~1 +- 0.3."""
    y = pool.tile([128, nt], F32, tag="nr_y")
    t = pool.tile([128, nt], F32, tag="nr_t")
    nc.vector.reciprocal(out=y, in_=veps)
    for _ in range(iters):
        nc.vector.tensor_mul(out=t, in0=y, in1=y)
        nc.vector.tensor_mul(out=t, in0=t, in1=veps)
        nc.vector.tensor_scalar(out=t, in0=t, scalar1=-0.5, scalar2=1.5,
                                op0=ALU.mult, op1=ALU.add)
        nc.vector.tensor_mul(out=y, in0=y, in1=t)
    return y


def _layernorm_to_bf16(nc, pool, x_sb, z_sb, nt, n0=0):
    """Per-token LN over the free dim (D=256). Stats + Newton rsqrt on DVE
    in groups of 4 token-tiles (so downstream consumers of early tiles can
    start before the whole sequence is reduced); the (x - mu) * rstd affine
    alternates between ScalarE (Identity w/ scale+bias APs) and VectorE
    (tensor_scalar w/ per-partition scalar APs)."""
    st6 = pool.tile([128, nt, 6], F32, tag="ln_st6")
    mv = pool.tile([128, nt, 2], F32, tag="ln_mv")
    veps = pool.tile([128, nt], F32, tag="ln_veps")
    nmr = pool.tile([128, nt], F32, tag="ln_nmr")
    ys = pool.tile([128, nt], F32, tag="ln_ys")
    for g4 in range(0, nt, 4):
        w = min(4, nt - g4)
        for n in range(g4, g4 + w):
            nc.vector.bn_stats(out=st6[:, n, :], in_=x_sb[:, n0 + n, :])
            nc.vector.bn_aggr(out=mv[:, n, :], in_=st6[:, n, :])
        nc.vector.tensor_scalar_add(out=veps[:, g4:g4 + w],
                                    in0=mv[:, g4:g4 + w, 1], scalar1=EPS)
        y = _newton_rsqrt(nc, pool, veps[:, g4:g4 + w], w)
        nc.vector.tensor_copy(out=ys[:, g4:g4 + w], in_=y)
        nc.vector.scalar_tensor_tensor(out=nmr[:, g4:g4 + w],
                                       in0=mv[:, g4:g4 + w, 0], scalar=-1.0,
                                       in1=y, op0=ALU.mult, op1=ALU.mult)
        for n in range(g4, g4 + w):
            if n % 2:
                nc.vector.tensor_scalar(out=z_sb[:, n0 + n, :],
                                        in0=x_sb[:, n0 + n, :],
                                        scalar1=ys[:, n:n + 1],
                                        scalar2=nmr[:, n:n + 1],
                                        op0=ALU.mult, op1=ALU.add)
            else:
                nc.scalar.activation(out=z_sb[:, n0 + n, :],
                                     in_=x_sb[:, n0 + n, :],
                                     func=AF.Identity, bias=nmr[:, n:n + 1],
                                     scale=ys[:, n:n + 1])


def _build_program():
    _patch_tile_drain()
    nc = bass.Bass()

    xkv = nc.dram_tensor("xkv", [S, D], F32, kind="ExternalInput")
    wqkvT = nc.dram_tensor("wqkvT", [128, 2, 3 * D], BF, kind="ExternalInput")
    wprojT = nc.dram_tensor("wprojT", [128, 2, D], BF, kind="ExternalInput")
    w1T = nc.dram_tensor("w1T", [128, 2, DFF], BF, kind="ExternalInput")
    w2T = nc.dram_tensor("w2T", [128, 4, D], BF, kind="ExternalInput")
    qbias = nc.dram_tensor("qbias", [128, 2], F32, kind="ExternalInput")
    bproj = nc.dram_tensor("bproj", [1, D], BF, kind="ExternalInput")
    b1 = nc.dram_tensor("b1", [128, 4], F32, kind="ExternalInput")
    b2 = nc.dram_tensor("b2", [1, D], BF, kind="ExternalInput")
    ident = nc.dram_tensor("ident", [128, 128], BF, kind="ExternalInput")
    ones_row = nc.dram_tensor("ones_row", [1, 512], BF, kind="ExternalInput")
    ones_col = nc.dram_tensor("ones_col", [128, 1], BF, kind="ExternalInput")
    bsel4 = nc.dram_tensor("bsel4", [4, 128], BF, kind="ExternalInput")
    zrow = nc.dram_tensor("zrow", [1, 128], BF, kind="ExternalInput")
    out_d = nc.dram_tensor("out", [SQ, D], F32, kind="ExternalOutput")

    with tile.TileContext(nc) as tc:
        with (
            tc.tile_pool(name="singles", bufs=1) as sg,
            tc.tile_pool(name="work", bufs=2) as wk,
            tc.tile_pool(name="attn", bufs=4) as atp,
            tc.tile_pool(name="psum", bufs=1, space="PSUM") as pp,
        ):
            # ---- persistent SBUF tensors
            x_sb = sg.tile([128, NKT, D], F32)          # full-seq x, token-major
            z1_sb = sg.tile([128, NKT, D], BF)
            zT = sg.tile([128, 2, S], BF)               # z1^T  [d, tok]
            qkT = sg.tile([128, 4, S], BF)              # Q^T (m 0-1), K^T (m 2-3)
            v_sb = sg.tile([128, NKT, D], BF)           # V token-major
            ctxT_u = sg.tile([128, 2, SQ], BF)          # unnormalized ctx^T
            ctxT = sg.tile([128, 2, SQ], BF)            # normalized ctx^T
            den_sb = sg.tile([128, 4, 512], BF)         # denominators per s-block
            denT_c = sg.tile([128, 64], F32)            # transposed-compact dens
            recT_c = sg.tile([128, 64], BF)             # 1/den (compact)
            rec_sb = sg.tile([4, 4, 512], BF)           # 1/den back in row form
            x2_sb = sg.tile([128, NQT, D], F32)
            z2_sb = sg.tile([128, NQT, D], BF)
            z2T = sg.tile([128, 2, SQ], BF)
            hT = sg.tile([128, 4, SQ], BF)
            out_sb = sg.tile([128, NQT, D], F32)

            wqkvT_sb = sg.tile([128, 2, 3 * D], BF)
            wprojT_sb = sg.tile([128, 2, D], BF)
            w1T_sb = sg.tile([128, 2, DFF], BF)
            w2T_sb = sg.tile([128, 4, D], BF)
            qbias_sb = sg.tile([128, 2], F32)
            bproj_sb = sg.tile([1, D], BF)
            b1_sb = sg.tile([128, 4], F32)
            b2_sb = sg.tile([1, D], BF)
            id_sb = sg.tile([128, 128], BF)
            onesr_sb = sg.tile([1, 512], BF)
            onesc_sb = sg.tile([128, 1], BF)
            bsel4_sb = sg.tile([4, 128], BF)
            zrow_sb = sg.tile([1, 128], BF)

            x_t = xkv.rearrange("(n p) d -> p n d", p=128)
            for n in range(NKT):
                eng = (nc.sync, nc.scalar, nc.gpsimd)[n % 3]
                eng.dma_start(out=x_sb[:, n, :], in_=x_t[:, n, :])

            for di, (dst, dsrc) in enumerate([
                (id_sb, ident), (wqkvT_sb, wqkvT), (wprojT_sb, wprojT),
                (w1T_sb, w1T), (w2T_sb, w2T), (qbias_sb, qbias),
                (bproj_sb, bproj), (b1_sb, b1), (b2_sb, b2),
                (onesr_sb, ones_row), (onesc_sb, ones_col),
                (bsel4_sb, bsel4), (zrow_sb, zrow),
            ]):
                eng = nc.sync if di % 2 == 0 else nc.scalar
                eng.dma_start(out=dst, in_=dsrc[:])

            phases = int(os.environ.get("K_PHASES", "99"))
            reps = int(os.environ.get("K_REPS", "1"))
            # PSUM tags: attention needs exactly 8 banks:
            #   sc0/sc1/sc2 (2 each, rotating score buffers), ctx (1), den (1).
            # Other phases reuse these slots for their rotating tiles.
            ptags = ["ctx", "den", "sc0", "sc1"]

            def _one_rep():
                # evacuation copies alternate between ScalarE and VectorE
                tgl = [0]

                def evac(out, in_):
                    if tgl[0] % 2:
                        nc.vector.tensor_copy(out=out, in_=in_)
                    else:
                        nc.scalar.copy(out=out, in_=in_)
                    tgl[0] += 1

                # ---- LN1 + transpose, pipelined per 4-token-tile group
                if phases < 1:
                    nc.sync.dma_start(out=out_d.rearrange("(n p) d -> p n d", p=128)[:, 0, :], in_=x_sb[:, 0, :])
                for nb in range(4 if phases >= 1 else 0):
                    _layernorm_to_bf16(nc, wk, x_sb, z1_sb, 4, n0=4 * nb)
                    if phases < 2:
                        continue
                    for c in range(2):
                        tp = pp.tile([128, 512], BF, tag=f"sc{(2 * nb + c) % 3}")
                        for k in range(4):
                            n = nb * 4 + k
                            nc.tensor.transpose(
                                tp[:, k * 128:(k + 1) * 128],
                                z1_sb[:, n, c * 128:(c + 1) * 128], id_sb)
                        evac(zT[:, c, nb * 512:(nb + 1) * 512], tp)

                # ---- Q^T / K^T (feature-major; no bias matmuls: Q-bias via
                # ACT evac, K-bias cancels in softmax exactly)
                i = 0
                ptags = ["ctx", "den", "sc0", "sc1", "sc2"]

                def emit_qk(m, qs):
                    nonlocal i
                    ps = pp.tile([128, 512], F32, tag=ptags[i % 5]); i += 1
                    for c in range(2):
                        nc.tensor.matmul(ps, lhsT=wqkvT_sb[:, c, m * 128:(m + 1) * 128],
                                         rhs=zT[:, c, qs:qs + 512],
                                         start=(c == 0), stop=(c == 1))
                    if m < 2:  # Q: fold bias on evacuation
                        nc.scalar.activation(out=qkT[:, m, qs:qs + 512], in_=ps,
                                             func=AF.Identity,
                                             bias=qbias_sb[:, m:m + 1], scale=1.0)
                    else:
                        evac(qkT[:, m, qs:qs + 512], ps)

                def emit_v(n):
                    nonlocal i
                    ps = pp.tile([128, D], F32, tag=ptags[i % 5],
                                 name="ps_v"); i += 1
                    for c in range(2):
                        nc.tensor.matmul(ps, lhsT=zT[:, c, n * 128:(n + 1) * 128],
                                         rhs=wqkvT_sb[:, c, 2 * D:3 * D],
                                         start=(c == 0), stop=(c == 1))
                    evac(v_sb[:, n, :], ps)

                if phases >= 3:
                    # K first (attention block 0 needs the full K^T), then Q
                    # for query-half 0, V, then the rest
                    for qs in range(0, S, 512):
                        emit_qk(2, qs)
                    emit_qk(0, 0)
                    for n in range(NKT):
                        emit_v(n)
                    emit_qk(0, 512)
                    for qs in range(0, S, 512):
                        emit_qk(3, qs)
                    emit_qk(1, 0)
                    emit_qk(1, 512)

                # ---- post-half chain, emitted as pieces interleaved into a
                # later attention block: softmax-normalize one query half,
                # proj + residual, LN2, z2 transpose, FFN1+SiLU, FFN2 +
                # residual + store.
                out_t = out_d.rearrange("(n p) d -> p n d", p=128)

                def post_half(qh):
                    nonlocal i
                    qs0 = qh * 512
                    # reciprocal of this half's compacted denominators
                    rc = recT_c[:, 32 * qh:32 * qh + 32]
                    with nc.allow_low_precision(
                            reason="1/denominator in bf16; scales a small "
                                   "residual delta of the output"):
                        nc.vector.reciprocal(
                            out=rc, in_=denT_c[:, 32 * qh:32 * qh + 32])
                    yield
                    for sb in (2 * qh, 2 * qh + 1):
                        for c in range(4):
                            col = sb * 16 + c * 4
                            tpr = pp.tile([4, 128], BF, tag=f"sc{c % 3}",
                                          name="tpr")
                            nc.tensor.transpose(
                                tpr, recT_c[:, col:col + 4], id_sb)
                            evac(rec_sb[:, sb, c * 128:(c + 1) * 128], tpr)
                        yield
                    for sb in (2 * qh, 2 * qh + 1):
                        g = sb % 2
                        rbc = pp.tile([128, 512], F32, tag=("ctx" if sb % 2 else "den"))
                        nc.tensor.matmul(rbc, lhsT=bsel4_sb,
                                         rhs=rec_sb[:, sb, :],
                                         start=True, stop=True)
                        nc.vector.tensor_tensor(ctxT[:, g, qs0:qs0 + 512],
                                                ctxT_u[:, g, qs0:qs0 + 512],
                                                rbc, ALU.mult)
                        yield
                    # proj + residual -> x2
                    for n in range(4 * qh, 4 * qh + 4):
                        ps = pp.tile([128, D], F32, tag=f"sc{n % 3}",
                                     name="ps_proj")
                        nc.tensor.matmul(ps, lhsT=onesr_sb[0:1, 0:128],
                                         rhs=bproj_sb[0:1, :], start=True,
                                         stop=False)
                        for c in range(2):
                            nc.tensor.matmul(ps, lhsT=ctxT[:, c, n * 128:(n + 1) * 128],
                                             rhs=wprojT_sb[:, c, :],
                                             start=False, stop=(c == 1))
                        nc.vector.tensor_tensor(x2_sb[:, n, :], ps,
                                                x_sb[:, n, :], ALU.add)
                        yield
                    # LN2 on this half's 4 token tiles
                    _layernorm_to_bf16(nc, wk, x2_sb, z2_sb, 4, n0=4 * qh)
                    yield
                    for c in range(2):
                        tp2 = pp.tile([128, 512], BF, tag=f"sc{c % 2}",
                                      name="tp2")
                        for k in range(4):
                            n = 4 * qh + k
                            nc.tensor.transpose(
                                tp2[:, k * 128:(k + 1) * 128],
                                z2_sb[:, n, c * 128:(c + 1) * 128], id_sb)
                        evac(z2T[:, c, qs0:qs0 + 512], tp2)
                        yield
                    # FFN1 (h^T = w1 @ z2^T), fused bias + SiLU on ScalarE
                    for m in range(4):
                        ps = pp.tile([128, 512], F32, tag=f"sc{m % 3}",
                                     name="ps_ffn1")
                        for c in range(2):
                            nc.tensor.matmul(ps, lhsT=w1T_sb[:, c, m * 128:(m + 1) * 128],
                                             rhs=z2T[:, c, qs0:qs0 + 512],
                                             start=(c == 0), stop=(c == 1))
                        nc.scalar.activation(out=hT[:, m, qs0:qs0 + 512],
                                             in_=ps, func=AF.Silu,
                                             bias=b1_sb[:, m:m + 1], scale=1.0)
                        yield
                    # FFN2 + residual -> out
                    for n in range(4 * qh, 4 * qh + 4):
                        ps = pp.tile([128, D], F32, tag=("ctx" if n % 2 else "den"),
                                     name="ps_ffn2")
                        nc.tensor.matmul(ps, lhsT=onesr_sb[0:1, 0:128],
                                         rhs=b2_sb[0:1, :], start=True,
                                         stop=False)
                        for c in range(4):
                            nc.tensor.matmul(ps, lhsT=hT[:, c, n * 128:(n + 1) * 128],
                                             rhs=w2T_sb[:, c, :], start=False,
                                             stop=(c == 3))
                        nc.vector.tensor_tensor(out_sb[:, n, :], ps,
                                                x2_sb[:, n, :], ALU.add)
                        deng = (nc.sync, nc.scalar, nc.gpsimd)[n % 3]
                        deng.dma_start(out=out_t[:, n, :],
                                       in_=out_sb[:, n, :])
                        yield

                # ---- attention: 4 super-blocks in qp-major order
                # ((qp0,g0), (qp0,g1), (qp1,g0), (qp1,g1)) so query-half 0's
                # entire post chain interleaves into half 1's attention.
                # Scores are head-PAIR tiles [128, 2, 512] (one psum bank per
                # head - concurrent row-tiled matmuls must hit distinct
                # banks), double-buffered; exp alternates ACT (true Exp) and
                # DVE (Schraudolph fast-exp); AV/den matmuls merge into one
                # 4-band wave per tile pair. ctx/den accumulate in one bank
                # each (col-tiled writes hit disjoint partitions - safe).
                expi = [0]
                F_ACT = float(os.environ.get("K_FACT", "0.57"))

                def emit_scores(g, qb0, tt, pair, buf):
                    sc = pp.tile([128, 2, 512], F32, tag=f"sc{buf}", name="sc")
                    for j in range(2):
                        h = 2 * pair + j
                        nc.tensor.matmul(
                            sc[:, j, :],
                            lhsT=qkT[32 * h:32 * (h + 1), 2 + g, tt * 128:(tt + 1) * 128],
                            rhs=qkT[32 * h:32 * (h + 1), g, qb0:qb0 + 512],
                            start=True, stop=True, tile_position=(32 * h, 0))
                    return sc

                def emit_exp(sc):
                    at = atp.tile([128, 2, 512], BF)
                    expi[0] += 1
                    if int(expi[0] * F_ACT) > int((expi[0] - 1) * F_ACT):
                        nc.scalar.activation(out=at, in_=sc, func=AF.Exp)
                    else:
                        nc.vector.tensor_scalar(out=at[:].bitcast(I16),
                                                in0=sc,
                                                scalar1=EXP_A, scalar2=EXP_B,
                                                op0=ALU.mult, op1=ALU.add)
                    return at

                pieces = []

                def run_piece():
                    while pieces:
                        try:
                            next(pieces[0])
                            return
                        except StopIteration:
                            pieces.pop(0)

                for qp in range(2 if phases >= 4 else 0):
                    for g in range(2):
                        sb = qp * 2 + g
                        ctx_ps = pp.tile([128, 512], F32, tag="ctx")
                        den_ps = pp.tile([128, 512], F32, tag="den")
                        nc.tensor.matmul(ctx_ps, lhsT=zrow_sb[0:1, :],
                                         rhs=onesr_sb[0:1, :], start=True,
                                         stop=False, skip_group_check=True)
                        nc.tensor.matmul(den_ps, lhsT=zrow_sb[0:1, :],
                                         rhs=onesr_sb[0:1, :], start=True,
                                         stop=False, skip_group_check=True)
                        qb0 = qp * 512
                        stream = [(tt, pair) for tt in range(NKT)
                                  for pair in range(2)]
                        pending = []
                        nbuf = 0
                        for k in range(2):
                            tt2, p2 = stream[k]
                            pending.append(emit_scores(g, qb0, tt2, p2, nbuf))
                            nbuf = (nbuf + 1) % 3
                        ats = {}
                        for k, (tt, pair) in enumerate(stream):
                            ats[k] = emit_exp(pending.pop(0))
                            if k + 2 < len(stream):
                                tt2, p2 = stream[k + 2]
                                pending.append(emit_scores(
                                    g, qb0, tt2, p2, nbuf))
                                nbuf = (nbuf + 1) % 3
                            if k % 2 == 0:
                                continue
                            # merged AV wave (4 col bands, one drain), then
                            # merged den wave (4 bands, permuted +2)
                            lastk = k == len(stream) - 1
                            for kk in (k - 1, k):
                                tt_, pair_ = stream[kk]
                                for j in range(2):
                                    h = 2 * pair_ + j
                                    h8 = 4 * g + h
                                    fin = lastk and kk == k and j == 1
                                    nc.tensor.matmul(
                                        ctx_ps[32 * h:32 * (h + 1), :],
                                        lhsT=v_sb[:, tt_, h8 * 32:(h8 + 1) * 32],
                                        rhs=ats[kk][:, j, :], start=False,
                                        stop=fin,
                                        tile_position=(0, 32 * h),
                                        skip_group_check=True)
                            for kk in (k - 1, k):
                                tt_, pair_ = stream[kk]
                                for j in range(2):
                                    h = 2 * pair_ + j
                                    hb = (h + 2) % 4
                                    fin = lastk and kk == k and j == 1
                                    nc.tensor.matmul(
                                        den_ps[32 * hb:32 * hb + 1, :],
                                        lhsT=onesc_sb[:, 0:1],
                                        rhs=ats[kk][:, j, :], start=False,
                                        stop=fin,
                                        tile_position=(0, 32 * hb),
                                        skip_group_check=True)
                            ats.clear()
                        # evacuate unnormalized ctx (DVE) + denominators (ACT)
                        nc.vector.tensor_copy(out=ctxT_u[:, g, qb0:qb0 + 512],
                                              in_=ctx_ps)
                        nc.scalar.copy(out=den_sb[:, sb, :], in_=den_ps)
                        # compact this super-block's dens now (PE transposes
                        # interleave with the next block's attention)
                        for c in range(4):
                            tpd = pp.tile([128, 128], BF,
                                          tag=f"sc{c % 3}", name="tpd")
                            nc.tensor.transpose(
                                tpd, den_sb[:, sb, c * 128:(c + 1) * 128],
                                id_sb)
                            col = sb * 16 + c * 4
                            nc.vector.tensor_copy(
                                out=denT_c[:, col:col + 4],
                                in_=tpd[:, 0:97:32])
                        if sb == 1:
                            pieces.append(post_half(0))
                    # end g loop
                if phases >= 4:
                    # preload the SiLU ACT table while the PE runs proj
                    # (walrus otherwise inserts the ~1.3us table load right
                    # before the first FFN1 SiLU)
                    dummy = wk.tile([1, 2], F32, tag="silu_warm")
                    nc.scalar.activation(out=dummy, in_=denT_c[0:1, 0:2],
                                         func=AF.Silu)
                    # run both halves' piece chains, staggered so one half's
                    # PE-heavy pieces overlap the other's DVE-heavy ones
                    gens = list(pieces)
                    pieces.clear()
                    for _ in range(7):
                        try:
                            next(gens[0])
                        except StopIteration:
                            gens.pop(0)
                            break
                    gens.append(post_half(1))
                    while gens:
                        for gen in list(gens):
                            try:
                                next(gen)
                            except StopIteration:
                                gens.remove(gen)
            for _rep in range(reps):
                _one_rep()

    if not int(os.environ.get("K_NO_WSPLIT", "0")):
        _split_excess_waits(nc)
    return nc


_PROGRAM = None
last_exec_time_ns = None
last_result = None


def _get_program():
    global _PROGRAM
    if _PROGRAM is None:
        _PROGRAM = _build_program()
    return _PROGRAM


def kernel(x, ln1_g, ln1_b, w_qkv, b_qkv, w_proj, b_proj,
           ln2_g, ln2_b, w1, b1, w2, b2):
    global last_exec_time_ns, last_result
    x = np.asarray(x, np.float32)
    ln1_g = np.asarray(ln1_g, np.float32)
    ln1_b = np.asarray(ln1_b, np.float32)
    w_qkv = np.asarray(w_qkv, np.float32)
    b_qkv = np.asarray(b_qkv, np.float32)
    w_proj = np.asarray(w_proj, np.float32)
    b_proj = np.asarray(b_proj, np.float32)
    ln2_g = np.asarray(ln2_g, np.float32)
    ln2_b = np.asarray(ln2_b, np.float32)
    w1 = np.asarray(w1, np.float32)
    b1 = np.asarray(b1, np.float32)
    w2 = np.asarray(w2, np.float32)
    b2 = np.asarray(b2, np.float32)

    scale = DH ** -0.5
    w_qkv_eff = w_qkv * ln1_g[None, :]
    b_qkv_eff = (b_qkv + w_qkv @ ln1_b).copy()
    w_qkv_eff[:D] *= scale
    b_qkv_eff[:D] *= scale
    # V-bias contributes b_v @ w_proj^T to the (normalized) attention
    # output — fold it into the projection bias (exact).
    b_proj_eff = b_proj + w_proj @ b_qkv_eff[2 * D:3 * D]
    # K-bias shifts every query's scores by a per-query constant -> cancels
    # in softmax; only the Q-side bias matters for the scores.
    qb = np.ascontiguousarray(b_qkv_eff[:D].reshape(2, 128).T).astype(np.float32)
    w1_eff = w1 * ln2_g[None, :]
    b1_eff = b1 + w1 @ ln2_b

    def fmt_T(w):  # [out_f, in_d] -> [128, in_chunks, out_f] bf16
        o, d = w.shape
        return np.ascontiguousarray(
            w.T.reshape(d // 128, 128, o).transpose(1, 0, 2)).astype(BF16)

    # den matmuls write row band (h+2)%4 (they use the pair's free PE col
    # bands); bsel4 maps band back to head for the broadcast matmul.
    bsel4 = np.zeros((4, 128), BF16)
    for m in range(128):
        bsel4[(m // 32 + 2) % 4, m] = 1

    shared = {
        "wqkvT": fmt_T(w_qkv_eff),
        "wprojT": fmt_T(w_proj),
        "w1T": fmt_T(w1_eff),
        "w2T": fmt_T(w2),
        "qbias": qb,
        "bproj": b_proj_eff[None, :].astype(BF16),
        "b1": np.ascontiguousarray(b1_eff.reshape(4, 128).T).astype(np.float32),
        "b2": b2[None, :].astype(BF16),
        "ident": np.eye(128, dtype=BF16),
        "ones_row": np.ones((1, 512), BF16),
        "ones_col": np.ones((128, 1), BF16),
        "bsel4": bsel4,
        "zrow": np.zeros((1, 128), BF16),
    }

    in_maps = []
    for c in range(8):
        b, hh = divmod(c, 2)
        xr = np.concatenate([x[b, hh * SQ:(hh + 1) * SQ],
                             x[b, (1 - hh) * SQ:(2 - hh) * SQ]], axis=0)
        m = dict(shared)
        m["xkv"] = np.ascontiguousarray(xr)
        in_maps.append(m)

    trace = os.environ.get("BASS_KERNEL_TRACE") == "1"
    res = run_bass_kernel_spmd(_get_program(), in_maps,
                               core_ids=list(range(8)), trace=trace)
    last_exec_time_ns = res.exec_time_ns
    last_result = res

    out = np.empty((B, S, D), np.float32)
    for c in range(8):
        b, hh = divmod(c, 2)
        out[b, hh * SQ:(hh + 1) * SQ] = res.results[c]["out"]
    return out
